# revision 20
# baseline (speedup 1.0000x reference)
"""Trainium2 Bass kernel for nn_CGLSTMEncoder (contextual-gate LSTM encoder).

Problem: x [32768, 1080] fp32 -> 294912 independent length-120 sequences
(9 vars folded into batch, D_in=1), LSTM cell H=32 with a contextual gate
replacing the output gate (the reference computes but never uses the o-gate).
Output: final hidden states [32768, 288] fp32.

Strategy (pure data parallel over 8 cores, 36864 rows/core):
 - Feature-on-partition layout, 4 row-chunks of 512 cols packed onto the
   128 partitions (supertile = 2048 rows). 18 supertiles/core, processed
   as 3 groups of 6 (3 supertile PAIRS per group, pipelined).
 - Per step, per gate-group q in [i, f, cg, g]: one block-diagonal K=128
   bf16 matmul (h-recurrence, same 32x32 weights for the 4 chunks) plus one
   K=5 matmul injecting x_t (4 chunk x-rows + ones row carrying the bias),
   accumulated in one PSUM bank [128, 512] per gate.
 - The g-gate weights are pre-doubled so tanh(g) = 2*sigmoid(2g) - 1; all
   four gates then activate in ONE sigmoid over [128, 2048] of PSUM. The
   kernel is ACT-engine bound (its per-column cost floor), so the remaining
   structure minimizes ACT work and stalls:
     * tanh(c) is batched per supertile PAIR ([128, 1024] per instruction,
       halving its fixed per-instruction access latency), and its emission
       lags the pair's cell update by two sigmoid slots so the in-order ACT
       queue never waits on the DVE chain.
     * the DVE chain is pair-batched too (t2, t1=i*t2, c=f*c, c+=t1,
       h=cg*tanh(c) on [128, 1024] tiles, bf16 in 2x mode, fp32 cell).
 - x is pre-transposed host-side into [9, 2, 4, 120, 512] bf16; 6-step
   slabs live in persistent per-supertile double buffers, prefetched one
   slab ahead. DMA issues are spread over the gpsimd/sync/scalar queues
   (weights on sync+scalar, outputs + ones on sync) so descriptor
   generation never serializes the t=0 fill or a group transition; the
   slab-1 prefetch is deferred off the fill, and a short dummy-matmul
   burst pre-warms the PE p-state during the first x DMA. Steady state
   runs exactly at the ACT-busy floor (14.47us per 6-supertile round);
   the residual ~24us is one-time fill/drain, transitions overlapped.
"""

import numpy as np
import ml_dtypes

SEQ, NV, H = 120, 9, 32
BATCH = 32768
NCORES = 8
BC = BATCH // NCORES      # 4096 batch rows per core
C = 512                   # columns per chunk (PSUM bank free size, fp32)
G4 = 4                    # chunks per supertile
HALVES = 2                # supertiles per var
IL = 6                    # interleaved supertiles (3 pairs)
NP = IL // 2              # pairs per group
S = 6                     # slab steps buffered per x DMA
BF16 = ml_dtypes.bfloat16

_cache = {}


def _build_weight_arrays(W_ih, W_hh, b_ih, b_hh, cg_w, cg_u, cg_b):
    # gate-bank order in PSUM: q0=i, q1=f, q2=cg, q3=g  (o-gate is unused)
    # q3 weights are doubled: tanh(g) is computed as 2*sigmoid(2g)-1.
    bias = b_ih + b_hh
    Ws = [W_hh[0:32], W_hh[32:64], cg_u, 2.0 * W_hh[64:96]]
    wxs = [W_ih[0:32, 0], W_ih[32:64, 0], cg_w[:, 0], 2.0 * W_ih[64:96, 0]]
    bs = [bias[0:32], bias[32:64], cg_b, 2.0 * bias[64:96]]
    LH = np.zeros((4, 128, 128), np.float32)
    LX = np.zeros((4, 5, 128), np.float32)
    for q in range(4):
        for g in range(G4):
            sl = slice(32 * g, 32 * g + 32)
            LH[q, sl, sl] = Ws[q].T          # [k, m]
            LX[q, g, sl] = wxs[q]
            LX[q, 4, sl] = bs[q]
    return LH.astype(BF16), LX.astype(BF16)


def _build_nc(n_v=NV, T=SEQ):
    import concourse.bacc as bacc
    import concourse.tile as tile
    from concourse import mybir

    AF = mybir.ActivationFunctionType
    ALU = mybir.AluOpType
    bf = mybir.dt.bfloat16
    f32 = mybir.dt.float32

    nc = bacc.Bacc("TRN2", target_bir_lowering=False, debug=False,
                   enable_asserts=False)
    xt_d = nc.dram_tensor("xt", [n_v, HALVES, G4, T, C], bf,
                          kind="ExternalInput")
    lh_d = nc.dram_tensor("lh", [4, 128, 128], bf, kind="ExternalInput")
    lx_d = nc.dram_tensor("lx", [4, 5, 128], bf, kind="ExternalInput")
    ones_d = nc.dram_tensor("ones", [1, S * C], bf, kind="ExternalInput")
    out_d = nc.dram_tensor("out", [n_v, HALVES, G4, 32, C], f32,
                           kind="ExternalOutput")
    xt, lh, lx, ones, out = (t.ap() for t in (xt_d, lh_d, lx_d, ones_d, out_d))

    stiles = [(v, hf) for v in range(n_v) for hf in range(HALVES)]
    n_slabs = (T + S - 1) // S

    with tile.TileContext(nc) as tc:
        with tc.tile_pool(name="w", bufs=1) as wp, \
             tc.tile_pool(name="x", bufs=2 * IL) as xp, \
             tc.tile_pool(name="ps", bufs=2, space="PSUM") as pp, \
             tc.tile_pool(name="sfc", bufs=2 * NP) as sp, \
             tc.tile_pool(name="tc", bufs=NP + 1) as tp, \
             tc.tile_pool(name="t2", bufs=NP) as vp, \
             tc.tile_pool(name="t1", bufs=NP) as up, \
             tc.tile_pool(name="st", bufs=2 * NP) as cp, \
             tc.tile_pool(name="ho", bufs=NP) as mp:

            lh_sb = wp.tile([128, 512], bf, tag="lh")
            lx_sb = wp.tile([5, 512], bf, tag="lx")
            # weights ride the sync/scalar queues so the first group's
            # slab-0 fetches start immediately on gpsimd
            for q in range(4):
                nc.sync.dma_start(lh_sb[:, 128 * q:128 * q + 128], lh[q])
                nc.scalar.dma_start(lx_sb[:, 128 * q:128 * q + 128], lx[q])

            # PE p-state pre-warm: ~3us of dummy matmuls during the initial
            # x-slab DMA wait so the first real matmuls run at full clock
            warm = wp.tile([128, 512], bf, tag="warm")
            nc.vector.memset(warm[:, :], 0.0)
            wP = pp.tile([128, G4, C], f32, tag="P", name="Pwarm")
            for _ in range(8):
                nc.tensor.matmul(wP[:, 0, :], warm[:, 0:128], warm[:, :],
                                 start=True, stop=True)

            for g0 in range(0, len(stiles), IL):
                group = stiles[g0:g0 + IL]
                sts = [dict() for _ in group]
                prs = [dict() for _ in range(NP)]
                pending = []

                def _dma_slab(k, m):
                    # fetch slab m for supertile k into parity buffer m%2;
                    # alternate issue queues so neither serializes the fill
                    d = sts[k]
                    v, hf = group[k]
                    xb = d["xb"][m % 2]
                    t0 = m * S
                    xq, oq = ((nc.gpsimd, nc.sync) if (k + m) % 2 == 0
                              else (nc.sync, nc.gpsimd))
                    xq.dma_start(xb[0:4, :], xt[v, hf, :, t0:t0 + S, :])
                    oq.dma_start(xb[4:5, :], ones[:, :])

                def _emit_mm_sig(k, t):
                    # slab mgmt + 8 matmuls + the pair-half sigmoid for
                    # supertile k at step t
                    d = sts[k]
                    v, hf = group[k]
                    j, half = divmod(k, 2)
                    col = (t % S) * C
                    if t == 0:
                        d["xb"] = [
                            xp.tile([5, S * C], bf, tag="xb",
                                    name=f"xb{k}_{p}")
                            for p in range(2)]
                        _dma_slab(k, 0)
                        if half == 0:
                            prs[j]["c"] = cp.tile([128, 2 * C], f32,
                                                  tag="c", name=f"c{j}")
                            prs[j]["h"] = cp.tile([128, 2 * C], bf,
                                                  tag="h", name=f"h{j}")
                    elif t == 1:
                        # slab-1 prefetch deferred off the critical t=0 fill
                        if n_slabs > 1:
                            _dma_slab(k, 1)
                    elif t % S == 0 and t // S + 1 < n_slabs:
                        _dma_slab(k, t // S + 1)
                    x5 = d["xb"][(t // S) % 2]
                    P = pp.tile([128, G4, C], f32, tag="P", name=f"P{k}")
                    hsl = prs[j]["h"][:, C * half:C * half + C]
                    for q in range(4):
                        if t > 0:
                            nc.tensor.matmul(
                                P[:, q, :],
                                lh_sb[:, 128 * q:128 * q + 128],
                                hsl, start=True, stop=False)
                        nc.tensor.matmul(
                            P[:, q, :],
                            lx_sb[:, 128 * q:128 * q + 128],
                            x5[:, col:col + C],
                            start=(t == 0), stop=True)
                    if half == 0:
                        prs[j]["sfc"] = sp.tile([128, G4, 2 * C], bf,
                                                tag="sfc", name=f"s{j}")
                    nc.scalar.activation(
                        prs[j]["sfc"][:, :, C * half:C * half + C],
                        P[:, :, :], AF.Sigmoid)

                def _emit_chain(j, t):
                    # DVE cell update for pair j at step t
                    pr = prs[j]
                    c = pr["c"]
                    i_p = pr["sfc"][:, 0, :]
                    f_p = pr["sfc"][:, 1, :]
                    s2g = pr["sfc"][:, 3, :]
                    t2 = vp.tile([128, 2 * C], bf, tag="t2", name=f"t2_{j}")
                    nc.vector.tensor_scalar(t2[:, :], s2g, 2.0, 1.0,
                                            ALU.mult, ALU.subtract)
                    if t == 0:
                        nc.vector.tensor_mul(c[:, :], i_p, t2[:, :])
                    else:
                        t1 = up.tile([128, 2 * C], bf, tag="t1",
                                     name=f"t1_{j}")
                        nc.vector.tensor_mul(t1[:, :], i_p, t2[:, :])
                        nc.vector.tensor_mul(c[:, :], f_p, c[:, :])
                        nc.vector.tensor_add(c[:, :], c[:, :], t1[:, :])

                def _emit_finish(j, t):
                    # tanh(c) [ACT, pair-batched] + h = cg*tanh(c) [DVE]
                    pr = prs[j]
                    cg_p = pr["sfc"][:, 2, :]
                    tct = tp.tile([128, 2 * C], bf, tag="tct",
                                  name=f"tct{j}")
                    nc.scalar.activation(tct[:, :], pr["c"][:, :], AF.Tanh)
                    if t < T - 1:
                        nc.vector.tensor_mul(pr["h"][:, :], cg_p, tct[:, :])
                    else:
                        ho = mp.tile([128, 2 * C], f32, tag="ho",
                                     name=f"ho{j}")
                        nc.vector.tensor_mul(ho[:, :], cg_p, tct[:, :])
                        for half in range(2):
                            v, hf = group[2 * j + half]
                            for g in range(G4):
                                nc.sync.dma_start(
                                    out[v, hf, g, :, :],
                                    ho[32 * g:32 * g + 32,
                                       C * half:C * half + C])

                for t in range(T):
                    for j in range(NP):
                        _emit_mm_sig(2 * j, t)
                        _emit_mm_sig(2 * j + 1, t)
                        # lagged finish: the pending pair's chain is >=2
                        # sigmoid slots old, so ACT never stalls on DVE
                        if pending:
                            _emit_finish(*pending.pop(0))
                        _emit_chain(j, t)
                        pending.append((j, t))
                for j, t in pending:
                    _emit_finish(j, t)
    nc.compile()
    return nc


def _prep_core_x(xc):
    # xc [BC, 1080] fp32 -> [9, 2, 4, 120, 512] bf16
    x3 = xc.reshape(BC, NV, SEQ)
    x5d = x3.reshape(HALVES, G4, C, NV, SEQ)
    return np.ascontiguousarray(x5d.transpose(3, 0, 1, 4, 2)).astype(BF16)


def _unpack_out(arr):
    # arr [9, 2, 4, 32, 512] f32 -> [BC, 288]
    return np.ascontiguousarray(
        arr.transpose(1, 2, 4, 0, 3)).reshape(BC, NV * H)


def _run(inputs, trace=False):
    from concourse.bass_utils import run_bass_kernel_spmd

    x = np.asarray(inputs["x"], np.float32)
    LH, LX = _build_weight_arrays(
        np.asarray(inputs["W_ih"], np.float32),
        np.asarray(inputs["W_hh"], np.float32),
        np.asarray(inputs["b_ih"], np.float32),
        np.asarray(inputs["b_hh"], np.float32),
        np.asarray(inputs["cg_w"], np.float32),
        np.asarray(inputs["cg_u"], np.float32),
        np.asarray(inputs["cg_b"], np.float32),
    )
    ones = np.ones((1, S * C), BF16)
    if "nc" not in _cache:
        _cache["nc"] = _build_nc()
    nc = _cache["nc"]
    in_maps = []
    for k in range(NCORES):
        in_maps.append({
            "xt": _prep_core_x(x[k * BC:(k + 1) * BC]),
            "lh": LH, "lx": LX, "ones": ones,
        })
    try:
        res = run_bass_kernel_spmd(nc, in_maps, core_ids=list(range(NCORES)),
                                   trace=trace)
    except ModuleNotFoundError:
        # no NTFF profiling hook in this environment; run untraced
        res = run_bass_kernel_spmd(nc, in_maps, core_ids=list(range(NCORES)),
                                   trace=False)
    except Exception:
        # transient NRT flakes (NRT_EXEC_UNIT_UNRECOVERABLE) clear on retry
        res = run_bass_kernel_spmd(nc, in_maps, core_ids=list(range(NCORES)),
                                   trace=False)
    out = np.concatenate(
        [_unpack_out(res.results[k]["out"]) for k in range(NCORES)], axis=0)
    return out, res


def kernel(**inputs):
    out, _ = _run(inputs, trace=False)
    return out


if __name__ == "__main__":
    nc = _build_nc(n_v=3, T=2 * S)
    print("built small nc ok")


# revision 29
# speedup vs baseline: 1.0015x; 1.0015x over previous
"""Trainium2 Bass kernel for nn_CGLSTMEncoder (contextual-gate LSTM encoder).

Problem: x [32768, 1080] fp32 -> 294912 independent length-120 sequences
(9 vars folded into batch, D_in=1), LSTM cell H=32 with a contextual gate
replacing the output gate (the reference computes but never uses the o-gate).
Output: final hidden states [32768, 288] fp32.

Strategy (pure data parallel over 8 cores, 36864 rows/core):
 - Feature-on-partition layout, 4 row-chunks of 512 cols packed onto the
   128 partitions (supertile = 2048 rows). 18 supertiles/core, processed
   as 3 groups of 6 (3 supertile PAIRS per group, pipelined).
 - Per step, per gate-group q in [i, f, cg, g]: one block-diagonal K=128
   bf16 matmul (h-recurrence, same 32x32 weights for the 4 chunks) plus one
   K=5 matmul injecting x_t (4 chunk x-rows + ones row carrying the bias),
   accumulated in one PSUM bank [128, 512] per gate.
 - The g-gate weights are pre-doubled so tanh(g) = 2*sigmoid(2g) - 1; all
   four gates then activate in ONE sigmoid over [128, 2048] of PSUM. The
   kernel is ACT-engine bound (its per-column cost floor), so the remaining
   structure minimizes ACT work and stalls:
     * tanh(c) is batched per supertile PAIR ([128, 1024] per instruction,
       halving its fixed per-instruction access latency), and its emission
       lags the pair's cell update by two sigmoid slots so the in-order ACT
       queue never waits on the DVE chain.
     * the DVE chain is pair-batched too (t2, t1=i*t2, c=f*c, c+=t1,
       h=cg*tanh(c) on [128, 1024] tiles, bf16 in 2x mode, fp32 cell).
 - x is pre-transposed host-side into [9, 2, 4, 120, 512] bf16; 6-step
   slabs live in persistent per-supertile double buffers, prefetched one
   slab ahead. DMA issues are spread over the gpsimd/sync/scalar queues
   (weights on sync+scalar, outputs + ones on sync) so descriptor
   generation never serializes the t=0 fill or a group transition; the
   slab-1 prefetch is deferred off the fill, and a short dummy-matmul
   burst pre-warms the PE p-state during the first x DMA. Steady state
   runs exactly at the ACT-busy floor (14.47us per 6-supertile round);
   the residual ~24us is one-time fill/drain, transitions overlapped.
"""

import numpy as np
import ml_dtypes

SEQ, NV, H = 120, 9, 32
BATCH = 32768
NCORES = 8
BC = BATCH // NCORES      # 4096 batch rows per core
C = 512                   # columns per chunk (PSUM bank free size, fp32)
G4 = 4                    # chunks per supertile
HALVES = 2                # supertiles per var
IL = 6                    # interleaved supertiles (3 pairs)
NP = IL // 2              # pairs per group
S = 6                     # slab steps buffered per x DMA
BF16 = ml_dtypes.bfloat16

_cache = {}


def _build_weight_arrays(W_ih, W_hh, b_ih, b_hh, cg_w, cg_u, cg_b):
    # gate-bank order in PSUM: q0=i, q1=f, q2=cg, q3=g  (o-gate is unused)
    # q3 weights are doubled: tanh(g) is computed as 2*sigmoid(2g)-1.
    bias = b_ih + b_hh
    Ws = [W_hh[0:32], W_hh[32:64], cg_u, 2.0 * W_hh[64:96]]
    wxs = [W_ih[0:32, 0], W_ih[32:64, 0], cg_w[:, 0], 2.0 * W_ih[64:96, 0]]
    bs = [bias[0:32], bias[32:64], cg_b, 2.0 * bias[64:96]]
    LH = np.zeros((4, 128, 128), np.float32)
    LX = np.zeros((4, 5, 128), np.float32)
    for q in range(4):
        for g in range(G4):
            sl = slice(32 * g, 32 * g + 32)
            LH[q, sl, sl] = Ws[q].T          # [k, m]
            LX[q, g, sl] = wxs[q]
            LX[q, 4, sl] = bs[q]
    # k-major [k, q, m] so each weight tile loads in ONE contiguous DMA
    LH = np.ascontiguousarray(LH.transpose(1, 0, 2))
    LX = np.ascontiguousarray(LX.transpose(1, 0, 2))
    return LH.astype(BF16), LX.astype(BF16)


def _build_nc(n_v=NV, T=SEQ):
    import concourse.bacc as bacc
    import concourse.tile as tile
    from concourse import mybir

    AF = mybir.ActivationFunctionType
    ALU = mybir.AluOpType
    bf = mybir.dt.bfloat16
    f32 = mybir.dt.float32

    nc = bacc.Bacc("TRN2", target_bir_lowering=False, debug=False,
                   enable_asserts=False)
    # xt row 4 of the 3rd dim is a baked-in ones row (bias injection), so
    # each slab fetch is a single DMA
    xt_d = nc.dram_tensor("xt", [n_v, HALVES, G4 + 1, T, C], bf,
                          kind="ExternalInput")
    lh_d = nc.dram_tensor("lh", [128, 4, 128], bf, kind="ExternalInput")
    lx_d = nc.dram_tensor("lx", [5, 4, 128], bf, kind="ExternalInput")
    out_d = nc.dram_tensor("out", [n_v, HALVES, G4, 32, C], f32,
                           kind="ExternalOutput")
    xt, lh, lx, out = (t.ap() for t in (xt_d, lh_d, lx_d, out_d))

    stiles = [(v, hf) for v in range(n_v) for hf in range(HALVES)]
    n_slabs = (T + S - 1) // S

    with tile.TileContext(nc) as tc:
        with tc.tile_pool(name="w", bufs=1) as wp, \
             tc.tile_pool(name="x", bufs=2 * IL) as xp, \
             tc.tile_pool(name="ps", bufs=2, space="PSUM") as pp, \
             tc.tile_pool(name="sfc", bufs=2 * NP) as sp, \
             tc.tile_pool(name="tc", bufs=NP + 1) as tp, \
             tc.tile_pool(name="t2", bufs=NP) as vp, \
             tc.tile_pool(name="t1", bufs=NP) as up, \
             tc.tile_pool(name="st", bufs=2 * NP) as cp, \
             tc.tile_pool(name="ho", bufs=NP) as mp:

            lh_sb = wp.tile([128, 512], bf, tag="lh")
            lx_sb = wp.tile([5, 512], bf, tag="lx")
            # weights ride the sync/scalar queues so the first group's
            # slab-0 fetches start immediately on gpsimd
            nc.sync.dma_start(lh_sb[:, :], lh[:, :, :])
            nc.scalar.dma_start(lx_sb[:, :], lx[:, :, :])

            # PE p-state pre-warm: ~3us of dummy matmuls during the initial
            # x-slab DMA wait so the first real matmuls run at full clock
            warm = wp.tile([128, 512], bf, tag="warm")
            nc.vector.memset(warm[:, :], 0.0)
            wP = pp.tile([128, G4, C], f32, tag="P", name="Pwarm")
            for _ in range(8):
                nc.tensor.matmul(wP[:, 0, :], warm[:, 0:128], warm[:, :],
                                 start=True, stop=True)

            for g0 in range(0, len(stiles), IL):
                group = stiles[g0:g0 + IL]
                sts = [dict() for _ in group]
                prs = [dict() for _ in range(NP)]
                pending = []

                def _dma_slab(k, m):
                    # fetch slab m for supertile k into parity buffer m%2;
                    # alternate issue queues so neither serializes the fill
                    d = sts[k]
                    v, hf = group[k]
                    xb = d["xb"][m % 2]
                    t0 = m * S
                    xq = nc.gpsimd if (k + m) % 2 == 0 else nc.sync
                    xq.dma_start(xb[0:5, :], xt[v, hf, :, t0:t0 + S, :])

                def _emit_mm_sig(k, t):
                    # slab mgmt + 8 matmuls + the pair-half sigmoid for
                    # supertile k at step t
                    d = sts[k]
                    v, hf = group[k]
                    j, half = divmod(k, 2)
                    col = (t % S) * C
                    if t == 0:
                        d["xb"] = [
                            xp.tile([5, S * C], bf, tag="xb",
                                    name=f"xb{k}_{p}")
                            for p in range(2)]
                        _dma_slab(k, 0)
                        if half == 0:
                            prs[j]["c"] = cp.tile([128, 2 * C], f32,
                                                  tag="c", name=f"c{j}")
                            prs[j]["h"] = cp.tile([128, 2 * C], bf,
                                                  tag="h", name=f"h{j}")
                    elif t == 1:
                        # slab-1 prefetch deferred off the critical t=0 fill
                        if n_slabs > 1:
                            _dma_slab(k, 1)
                    elif t % S == 0 and t // S + 1 < n_slabs:
                        _dma_slab(k, t // S + 1)
                    x5 = d["xb"][(t // S) % 2]
                    P = pp.tile([128, G4, C], f32, tag="P", name=f"P{k}")
                    hsl = prs[j]["h"][:, C * half:C * half + C]
                    for q in range(4):
                        if t > 0:
                            nc.tensor.matmul(
                                P[:, q, :],
                                lh_sb[:, 128 * q:128 * q + 128],
                                hsl, start=True, stop=False)
                        nc.tensor.matmul(
                            P[:, q, :],
                            lx_sb[:, 128 * q:128 * q + 128],
                            x5[:, col:col + C],
                            start=(t == 0), stop=True)
                    if half == 0:
                        prs[j]["sfc"] = sp.tile([128, G4, 2 * C], bf,
                                                tag="sfc", name=f"s{j}")
                    nc.scalar.activation(
                        prs[j]["sfc"][:, :, C * half:C * half + C],
                        P[:, :, :], AF.Sigmoid)

                def _emit_chain(j, t):
                    # DVE cell update for pair j at step t
                    pr = prs[j]
                    c = pr["c"]
                    i_p = pr["sfc"][:, 0, :]
                    f_p = pr["sfc"][:, 1, :]
                    s2g = pr["sfc"][:, 3, :]
                    t2 = vp.tile([128, 2 * C], bf, tag="t2", name=f"t2_{j}")
                    nc.vector.tensor_scalar(t2[:, :], s2g, 2.0, 1.0,
                                            ALU.mult, ALU.subtract)
                    if t == 0:
                        nc.vector.tensor_mul(c[:, :], i_p, t2[:, :])
                    else:
                        t1 = up.tile([128, 2 * C], bf, tag="t1",
                                     name=f"t1_{j}")
                        nc.vector.tensor_mul(t1[:, :], i_p, t2[:, :])
                        nc.vector.tensor_mul(c[:, :], f_p, c[:, :])
                        nc.vector.tensor_add(c[:, :], c[:, :], t1[:, :])

                def _emit_finish(j, t):
                    # tanh(c) [ACT, pair-batched] + h = cg*tanh(c) [DVE]
                    pr = prs[j]
                    cg_p = pr["sfc"][:, 2, :]
                    tct = tp.tile([128, 2 * C], bf, tag="tct",
                                  name=f"tct{j}")
                    nc.scalar.activation(tct[:, :], pr["c"][:, :], AF.Tanh)
                    if t < T - 1:
                        nc.vector.tensor_mul(pr["h"][:, :], cg_p, tct[:, :])
                    else:
                        ho = mp.tile([128, 2 * C], f32, tag="ho",
                                     name=f"ho{j}")
                        nc.vector.tensor_mul(ho[:, :], cg_p, tct[:, :])
                        for half in range(2):
                            v, hf = group[2 * j + half]
                            # DRAM layout is linear in (chunk, feat): one DMA
                            nc.sync.dma_start(out[v, hf],
                                              ho[:, C * half:C * half + C])

                for t in range(T):
                    for j in range(NP):
                        _emit_mm_sig(2 * j, t)
                        _emit_mm_sig(2 * j + 1, t)
                        # lagged finish: the pending pair's chain is >=2
                        # sigmoid slots old, so ACT never stalls on DVE
                        if pending:
                            _emit_finish(*pending.pop(0))
                        _emit_chain(j, t)
                        pending.append((j, t))
                for j, t in pending:
                    _emit_finish(j, t)
    nc.compile()
    return nc


def _prep_core_x(xc):
    # xc [BC, 1080] fp32 -> [9, 2, 5, 120, 512] bf16 (row 4 = ones)
    x3 = xc.reshape(BC, NV, SEQ)
    x5d = x3.reshape(HALVES, G4, C, NV, SEQ)
    xt = np.empty((NV, HALVES, G4 + 1, SEQ, C), BF16)
    xt[:, :, :G4] = x5d.transpose(3, 0, 1, 4, 2).astype(BF16)
    xt[:, :, G4] = BF16(1.0)
    return xt


def _unpack_out(arr):
    # arr [9, 2, 4, 32, 512] f32 -> [BC, 288]
    return np.ascontiguousarray(
        arr.transpose(1, 2, 4, 0, 3)).reshape(BC, NV * H)


def _run(inputs, trace=False):
    from concourse.bass_utils import run_bass_kernel_spmd

    x = np.asarray(inputs["x"], np.float32)
    LH, LX = _build_weight_arrays(
        np.asarray(inputs["W_ih"], np.float32),
        np.asarray(inputs["W_hh"], np.float32),
        np.asarray(inputs["b_ih"], np.float32),
        np.asarray(inputs["b_hh"], np.float32),
        np.asarray(inputs["cg_w"], np.float32),
        np.asarray(inputs["cg_u"], np.float32),
        np.asarray(inputs["cg_b"], np.float32),
    )
    if "nc" not in _cache:
        _cache["nc"] = _build_nc()
    nc = _cache["nc"]
    in_maps = []
    for k in range(NCORES):
        in_maps.append({
            "xt": _prep_core_x(x[k * BC:(k + 1) * BC]),
            "lh": LH, "lx": LX,
        })
    try:
        res = run_bass_kernel_spmd(nc, in_maps, core_ids=list(range(NCORES)),
                                   trace=trace)
    except ModuleNotFoundError:
        # no NTFF profiling hook in this environment; run untraced
        res = run_bass_kernel_spmd(nc, in_maps, core_ids=list(range(NCORES)),
                                   trace=False)
    except Exception:
        # transient NRT flakes (NRT_EXEC_UNIT_UNRECOVERABLE) clear on retry
        res = run_bass_kernel_spmd(nc, in_maps, core_ids=list(range(NCORES)),
                                   trace=False)
    out = np.concatenate(
        [_unpack_out(res.results[k]["out"]) for k in range(NCORES)], axis=0)
    return out, res


def kernel(**inputs):
    out, _ = _run(inputs, trace=False)
    return out


if __name__ == "__main__":
    nc = _build_nc(n_v=3, T=2 * S)
    print("built small nc ok")
